# revision 1
# baseline (speedup 1.0000x reference)
"""Trainium2 Bass kernel for an 8-expert top-2 MoE (SwiGLU experts).

Problem shapes: T=256 tokens, H=1024 hidden, I=4096 intermediate,
E=8 experts, top_k=2, fp32.

Strategy (expert parallel over 8 NeuronCores):
  - Core c holds expert c's weights (w1s[c], w2s[c], w3s[c]): 48 MiB fp32.
  - The router (gate matmul + softmax + top-2 + renormalize) is replicated
    on every core; the gate matrix is fed with its columns rotated per-core
    so that column 0 is always the core's own expert (top-k/softmax are
    permutation-invariant, so the routing weights are unchanged).
  - Each core computes its expert's SwiGLU MLP densely over all 256 tokens
    in "transposed" activation layout (feature on partitions, token on the
    free axis) so the weight matrices are consumed directly as the matmul
    stationary operand with zero on-device transposes; hidden_states is fed
    pre-transposed ([H, T]) from the host.
  - The per-token combine weight for the core's expert (0 for tokens that
    didn't select it) scales the expert output; an on-device ReduceScatter
    over the 8 cores sums the partials (the arithmetic of the source model's
    tensor_model_parallel_all_reduce), leaving token shard c on core c; the
    host concatenates the 8 shards into the full [T, H] output.

The three big matmuls use the fp32r datapath (full-rate fp32 matmul with
relaxed mantissa, ~1.5e-4 relative error); the router matmul runs in exact
fp32 so top-2 expert selection bit-matches a reference fp32 router.

This is a memory-bound problem: each core must stream 48 MiB of expert
weights from HBM (~140 us at ~360 GB/s); the PE work (~82 us fp32r) and
everything else hides under the weight DMA.
"""

import sys

if "/opt/trn_rl_repo" not in sys.path:
    sys.path.insert(0, "/opt/trn_rl_repo")

import numpy as np

import concourse.bacc as bacc
import concourse.mybir as mybir
import concourse.tile as tile
from concourse.bass import ds as bass_ds, ts
from concourse.bass_utils import run_bass_kernel_spmd

T, H, I, E = 256, 1024, 4096, 8
N_CORES = 8
HK = H // 128  # 8 h-chunks (contraction for w1/w3)
MK = I // 128  # 32 i-chunks (psum/partition chunks of the intermediate)
GROUPS = 8  # w1/w3 weight-staging groups along I
MPG = MK // GROUPS  # 4 i-chunks per group
IG = I // GROUPS  # 512 intermediate columns per group
# W2 staging stages (i-chunks each): small first stages so the first W2
# matmul chain's weights land early in the SP DMA FIFO; 4 MB steady-state.
W2_STAGES = (4, 4, 4, 4, 4, 4, 4, 4)
W2_START = (0, 4, 8, 12, 16, 20, 24, 28)
W2_STAGE_OF = sum(([s] * n for s, n in enumerate(W2_STAGES)), [])
TK = T // 128  # 2 token chunks
NH = H // 512  # 2 psum halves of the output's H axis

F32 = mybir.dt.float32
F32R = mybir.dt.float32r
AF = mybir.ActivationFunctionType
ALU = mybir.AluOpType
AX = mybir.AxisListType


def build_nc(
    iters: int = 1,
    n_cores: int = N_CORES,
    with_collective: bool = True,
    silu_native: bool = True,
    debug_comb: bool = False,
    combine: str = "rs",
    router_bitcast: bool = True,
):
    """Build the SPMD program. `iters` repeats the whole compute body (for
    steady-state timing); the collective + output store run once at the end.
    `silu_native=False` lowers silu as sigmoid+mul (CoreSim has no Silu).
    `combine`: "rs" = on-device ReduceScatter (output is this core's [T/8, H]
    token shard; host concatenates), "ar" = on-device AllReduce (full output
    on every core). `router_bitcast`: feed the router matmul from the fp32r
    activation tile via bitcast instead of a separate fp32 copy of x^T."""
    nc = bacc.Bacc("TRN2", target_bir_lowering=False, debug=False, num_devices=n_cores)

    xTr = nc.dram_tensor("xTr", [H, T], F32R, kind="ExternalInput")
    if not router_bitcast:
        xT32 = nc.dram_tensor("xT32", [H, T], F32, kind="ExternalInput")
    gate = nc.dram_tensor("gate", [H, E], F32, kind="ExternalInput")
    w1 = nc.dram_tensor("w1", [H, I], F32R, kind="ExternalInput")
    w2 = nc.dram_tensor("w2", [I, H], F32R, kind="ExternalInput")
    w3 = nc.dram_tensor("w3", [H, I], F32R, kind="ExternalInput")
    TS = T // n_cores  # output token-shard rows under ReduceScatter
    if combine == "rs" and with_collective:
        out = nc.dram_tensor("out", [TS, H], F32, kind="ExternalOutput")
    else:
        out = nc.dram_tensor("out", [T, H], F32, kind="ExternalOutput")
    if debug_comb:
        combdbg = nc.dram_tensor("combdbg", [T, 1], F32, kind="ExternalOutput")
        pdbg = nc.dram_tensor("pdbg", [T, E], F32, kind="ExternalOutput")

    # DRAM views with a 128-partition inner dim for DMA into SBUF tiles.
    xTr_v = xTr.ap().rearrange("(ho hi) t -> hi ho t", hi=128)  # [128, 8, 256]
    if not router_bitcast:
        xT32_v = xT32.ap().rearrange("(ho hi) t -> hi ho t", hi=128)
    gate_v = gate.ap().rearrange("(ho hi) e -> hi ho e", hi=128)  # [128, 8, 8]
    w1_v = w1.ap().rearrange("(ho hi) i -> hi ho i", hi=128)  # [128, 8, 4096]
    w3_v = w3.ap().rearrange("(ho hi) i -> hi ho i", hi=128)
    w2_v = w2.ap().rearrange("(ko ki) h -> ki ko h", ki=128)  # [128, 32, 1024]

    with tile.TileContext(nc) as tc:
        with (
            tc.tile_pool(name="zpool", bufs=2) as zpool,
            tc.tile_pool(name="w1p", bufs=3) as w1p,
            tc.tile_pool(name="w3p", bufs=3) as w3p,
            tc.tile_pool(name="w2p", bufs=2) as w2p,
            tc.tile_pool(name="hpool", bufs=4) as hpool,
            tc.tile_pool(name="small", bufs=2) as small,
            tc.tile_pool(name="outsb", bufs=2) as outsb,
            tc.tile_pool(name="ps_h1", bufs=2, space="PSUM") as ps_h1,
            tc.tile_pool(name="ps_h3", bufs=2, space="PSUM") as ps_h3,
            tc.tile_pool(name="ps_out", bufs=1, space="PSUM") as ps_out,
            tc.tile_pool(name="dram", bufs=1, space="DRAM") as dram,
        ):
            partial = dram.tile([T, H], F32)  # collective input bounce
            if combine == "rs":
                reduced = dram.tile([TS, H], F32)  # ReduceScatter output bounce
            else:
                reduced = dram.tile([T, H], F32)  # AllReduce output bounce

            def body(_iv=None):
                # ---- activations + gate (fresh from DRAM each iteration)
                z = zpool.tile([128, HK, T], F32R, tag="z")
                g_sb = zpool.tile([128, HK, E], F32, tag="g")
                nc.gpsimd.dma_start(z[:], xTr_v)
                nc.gpsimd.dma_start(g_sb[:], gate_v)
                if router_bitcast:
                    z32 = z.bitcast(F32)
                else:
                    z32 = zpool.tile([128, HK, T], F32, tag="z32")
                    nc.gpsimd.dma_start(z32[:], xT32_v)

                # ---- router: logits -> softmax -> top-2 renormalized weight
                # for THIS core's expert (gate column 0). comb0[t] is a
                # [128,1] per-token scale, 0 when the token skips this expert.
                comb0 = []
                for t in range(TK):
                    ps_r = ps_h1.tile([128, E], F32, tag="h1")
                    for hk in range(HK):
                        nc.tensor.matmul(
                            ps_r[:],
                            z32[:, hk, ts(t, 128)],
                            g_sb[:, hk, :],
                            start=(hk == 0),
                            stop=(hk == HK - 1),
                        )
                    neg_mx = small.tile([128, 1], F32, tag="neg_mx")
                    nc.vector.tensor_reduce(
                        neg_mx[:], ps_r[:], AX.X, ALU.max, negate=True
                    )
                    ex = small.tile([128, E], F32, tag="ex")
                    nc.scalar.activation(ex[:], ps_r[:], AF.Exp, bias=neg_mx[:])
                    ssum = small.tile([128, 1], F32, tag="ssum")
                    nc.vector.tensor_reduce(ssum[:], ex[:], AX.X, ALU.add)
                    srec = small.tile([128, 1], F32, tag="srec")
                    nc.vector.reciprocal(srec[:], ssum[:])
                    p = small.tile([128, E], F32, tag="p")
                    nc.vector.tensor_scalar_mul(p[:], ex[:], srec[:])
                    m1 = small.tile([128, 1], F32, tag="m1")
                    nc.vector.tensor_reduce(m1[:], p[:], AX.X, ALU.max)
                    # knock out the top-1 entry, then the max of the rest is top-2
                    pm = small.tile([128, E], F32, tag="pm")
                    nc.vector.tensor_single_scalar(pm[:], p[:], m1[:], ALU.is_equal)
                    p2 = small.tile([128, E], F32, tag="p2")
                    nc.vector.scalar_tensor_tensor(
                        p2[:], pm[:], -2.0, p[:], ALU.mult, ALU.add
                    )
                    m2 = small.tile([128, 1], F32, tag="m2")
                    nc.vector.tensor_reduce(m2[:], p2[:], AX.X, ALU.max)
                    denom = small.tile([128, 1], F32, tag="denom")
                    nc.vector.tensor_add(denom[:], m1[:], m2[:])
                    drec = small.tile([128, 1], F32, tag="drec")
                    nc.vector.reciprocal(drec[:], denom[:])
                    sel = small.tile([128, 1], F32, tag="sel")
                    nc.vector.tensor_single_scalar(
                        sel[:], p[:, 0:1], m2[:], ALU.is_ge
                    )
                    wn = small.tile([128, 1], F32, tag="wn")
                    nc.vector.tensor_scalar_mul(wn[:], p[:, 0:1], drec[:])
                    cb = small.tile([128, 1], F32, tag="cb")
                    nc.vector.tensor_mul(cb[:], wn[:], sel[:])
                    comb0.append(cb)
                    if debug_comb:
                        nc.sync.dma_start(combdbg[ts(t, 128), :], cb[:])
                        nc.sync.dma_start(pdbg[ts(t, 128), :], p[:])

                # ---- expert MLP, transposed layout, grouped weight streaming
                out_ps = [
                    ps_out.tile([128, H], F32, tag=f"out{t}", name=f"out_ps{t}")
                    for t in range(TK)
                ]
                w1_sb = w3_sb = None
                hm_tiles = [None] * MK
                w2_sbs = {}

                def w2_chain(m):
                    s = W2_STAGE_OF[m]
                    off = m - W2_START[s]
                    for t in range(TK):
                        for n in range(NH):
                            nc.tensor.matmul(
                                out_ps[t][:, ts(n, 512)],
                                hm_tiles[m][:, ts(t, 128)],
                                w2_sbs[s][:, off, ts(n, 512)],
                                start=(m == 0),
                                stop=(m == MK - 1),
                            )

                def stage_w2(m):
                    s = W2_STAGE_OF[m]
                    if m != W2_START[s]:
                        return
                    nch = W2_STAGES[s]
                    w2_sbs[s] = w2p.tile(
                        [128, nch, H], F32R, tag="w2", name=f"w2sb{s}"
                    )
                    nc.sync.dma_start(
                        w2_sbs[s][:], w2_v[:, bass_ds(W2_START[s], nch), :]
                    )

                for m in range(MK):
                    g, kk = divmod(m, MPG)
                    # first W2 stage goes ahead of w1/w3 in the DMA FIFO so the
                    # first W2 matmul chain never head-of-line-blocks PE
                    stage_w2(m)
                    if kk == 0:
                        w1_sb = w1p.tile([128, HK, IG], F32R, tag="w1")
                        w3_sb = w3p.tile([128, HK, IG], F32R, tag="w3")
                        nc.sync.dma_start(w1_sb[:], w1_v[:, :, ts(g, IG)])
                        nc.sync.dma_start(w3_sb[:], w3_v[:, :, ts(g, IG)])
                    h1m = ps_h1.tile([128, T], F32, tag="h1")
                    h3m = ps_h3.tile([128, T], F32, tag="h3")
                    for hk in range(HK):
                        nc.tensor.matmul(
                            h1m[:],
                            w1_sb[:, hk, ts(kk, 128)],
                            z[:, hk, :],
                            start=(hk == 0),
                            stop=(hk == HK - 1),
                        )
                    for hk in range(HK):
                        nc.tensor.matmul(
                            h3m[:],
                            w3_sb[:, hk, ts(kk, 128)],
                            z[:, hk, :],
                            start=(hk == 0),
                            stop=(hk == HK - 1),
                        )
                    h1s = hpool.tile([128, T], F32, tag="h1s")
                    if silu_native:
                        nc.scalar.activation(h1s[:], h1m[:], AF.Silu)
                    else:
                        sg = hpool.tile([128, T], F32, tag="sg")
                        nc.scalar.activation(sg[:], h1m[:], AF.Sigmoid)
                        nc.vector.tensor_mul(h1s[:], sg[:], h1m[:])
                    hm = hpool.tile([128, T], F32R, tag="hm")
                    nc.vector.tensor_mul(hm[:], h1s[:], h3m[:])
                    hm_tiles[m] = hm
                    # W2 for the previous i-chunk: gives ACT/DVE one chunk of
                    # slack to produce hm before PE needs it.
                    if m >= 1:
                        w2_chain(m - 1)
                w2_chain(MK - 1)

                # ---- scale by this expert's combine weight, store partial
                for t in range(TK):
                    o_sb = outsb.tile([128, H], F32, tag=f"o{t}")
                    nc.vector.tensor_scalar_mul(o_sb[:], out_ps[t][:], comb0[t][:])
                    nc.gpsimd.dma_start(partial[ts(t, 128), :], o_sb[:])

            if iters == 1:
                body()
            else:
                with tc.For_i(
                    0, iters, 1, hint_engines=(mybir.EngineType.PE,)
                ) as iv:
                    body(iv)

            if with_collective:
                nc.gpsimd.collective_compute(
                    "ReduceScatter" if combine == "rs" else "AllReduce",
                    ALU.add,
                    replica_groups=[list(range(n_cores))],
                    ins=[partial[:].opt()],
                    outs=[reduced[:].opt()],
                )
                nc.sync.dma_start(out[:], reduced[:])
            else:
                nc.sync.dma_start(out[:], partial[:])

    nc.compile()
    return nc


_CACHE = {}


def _built(key):
    if key not in _CACHE:
        _CACHE[key] = build_nc(*key)
    return _CACHE[key]


def make_in_maps(
    hidden_states, gate_w, w1s, w2s, w3s, n_cores=N_CORES, router_bitcast=True
):
    xT = np.ascontiguousarray(np.asarray(hidden_states, dtype=np.float32).T)
    gate_w = np.asarray(gate_w, dtype=np.float32)
    w1s = np.asarray(w1s, dtype=np.float32)
    w2s = np.asarray(w2s, dtype=np.float32)
    w3s = np.asarray(w3s, dtype=np.float32)
    in_maps = []
    for c in range(n_cores):
        m = {
            "xTr": xT,
            # rotate gate columns so column 0 is this core's expert
            "gate": np.ascontiguousarray(np.roll(gate_w, -c, axis=1)),
            "w1": np.ascontiguousarray(w1s[c]),
            "w2": np.ascontiguousarray(w2s[c]),
            "w3": np.ascontiguousarray(w3s[c]),
        }
        if not router_bitcast:
            m["xT32"] = xT
        in_maps.append(m)
    return in_maps


def kernel(hidden_states, gate_w, w1s, w2s, w3s):
    in_maps = make_in_maps(hidden_states, gate_w, w1s, w2s, w3s)
    nc = _built((1, N_CORES, True))
    res = run_bass_kernel_spmd(nc, in_maps, core_ids=list(range(N_CORES)))
    # ReduceScatter leaves token shard c on core c; concatenate the shards.
    return np.concatenate(
        [np.asarray(res.results[c]["out"]) for c in range(N_CORES)], axis=0
    ).astype(np.float32, copy=False)

